# revision 1
# baseline (speedup 1.0000x reference)
"""Trainium2 Bass kernel for nn_MultiHeadAttention_59614146068609.

Sharding: 8 cores = 2 batches x 4 head-groups (4 heads each).
Each core projects q/k/v for its batch with its head-slice of Wq/Wk/Wv
(column-sharded), runs causal+padded attention for its 4 heads, and
applies its row-slice of Wo, producing a partial [D, S] output. The host
sums the 4 partials per batch and adds bo.

All matmuls run as float32r (2 cycles/row PE mode, ~1e-4 rel err).
Layout trick: scores are computed transposed (S.T[k, q], k on
partitions) so softmax sums come from an appended ones-column of V and
no on-chip transposes are needed anywhere.

The kernel is specialized at build time on kb_cap = number of 128-wide
key blocks that contain any unpadded key (derived from the runtime
key_padding_mask); fully padded key blocks contribute exactly zero
attention weight, so their projection/QK/exp/PV work is skipped.
"""

import numpy as np

S = 2048
B = 2
D = 1024
H = 16
DK = 64
N_CORES = 8
GROUPS = N_CORES // B          # head groups per batch = 4
HPG = H // GROUPS              # heads per group = 4
OC = HPG * DK                  # per-core projected dim = 256
OT = OC // 128                 # o-tiles per core = 2
IT = D // 128                  # contraction tiles = 8
SC = S // 512                  # sequence chunks of 512 = 4
KB = S // 128                  # k blocks of 128 = 16
NEG = -1e30

_cache = {}


def _build_nc(kb_cap):
    import concourse.bacc as bacc
    import concourse.bass as bass
    import concourse.mybir as mybir
    import concourse.tile as tile
    from concourse import library_config

    F32 = mybir.dt.float32
    F32R = mybir.dt.float32r
    FP16 = mybir.dt.float16
    Exp = mybir.ActivationFunctionType.Exp
    Identity = mybir.ActivationFunctionType.Identity
    PSUM = bass.MemorySpace.PSUM

    ksc = -(-kb_cap * 128 // 512)        # 512-chunks of k_T to project
    vrounds = [
        range(r * 8, min(kb_cap, (r + 1) * 8)) for r in range(-(-kb_cap // 8))
    ]

    nc = bacc.Bacc("TRN2", target_bir_lowering=False, debug=False)

    xq = nc.dram_tensor("xq", [D, S], FP16, kind="ExternalInput")
    xk = nc.dram_tensor("xk", [D, S], FP16, kind="ExternalInput")
    xv = nc.dram_tensor("xv", [D, S], FP16, kind="ExternalInput")
    wq = nc.dram_tensor("wq", [D, OC], FP16, kind="ExternalInput")
    wk = nc.dram_tensor("wk", [D, OC], FP16, kind="ExternalInput")
    wv = nc.dram_tensor("wv", [D, OC], FP16, kind="ExternalInput")
    wo = nc.dram_tensor("wo", [OC, D], FP16, kind="ExternalInput")
    bias_qk = nc.dram_tensor("bias_qk", [128, 2, OT], F32, kind="ExternalInput")
    bias_v = nc.dram_tensor("bias_v", [1, OC], F32, kind="ExternalInput")
    pad = nc.dram_tensor("pad", [128, KB], F32, kind="ExternalInput")
    causal = nc.dram_tensor("causal", [128, 128], FP16, kind="ExternalInput")
    out_t = nc.dram_tensor("out_t", [D, S], F32, kind="ExternalOutput")

    with tile.TileContext(nc) as tc, nc.allow_low_precision(
        reason="fp32r compute throughout; validated vs fp64 reference"
    ):
        with (
            tc.tile_pool(name="persist", bufs=1) as pp,
            tc.tile_pool(name="xs", bufs=6) as xs,
        ):
            nc.gpsimd.load_library(library_config.attn)

            # ---- persistent SBUF tensors ----
            t_wq = pp.tile([128, IT, OC], FP16)
            t_wk = pp.tile([128, IT, OC], FP16)
            t_wv = pp.tile([128, IT, OC], FP16)
            t_wo = pp.tile([128, OT, D], FP16)
            t_bqk = pp.tile([128, 2, OT], F32)
            t_bv = pp.tile([128, OC], F32)
            t_pad = pp.tile([128, KB], F32)
            t_causal = pp.tile([128, 128], FP16)
            t_qT = pp.tile([128, HPG, S], FP16)
            t_kT = pp.tile([128, HPG, ksc * 512], FP16)
            t_V = pp.tile([128, kb_cap, HPG, 128], FP16)
            t_OT = pp.tile([128, OT, S], FP16)

            nc.scalar.dma_start(out=t_wq, in_=wq[:].rearrange("(i p) o -> p i o", p=128))
            nc.scalar.dma_start(out=t_wk, in_=wk[:].rearrange("(i p) o -> p i o", p=128))
            nc.scalar.dma_start(out=t_wv, in_=wv[:].rearrange("(i p) o -> p i o", p=128))
            nc.scalar.dma_start(out=t_bqk, in_=bias_qk[:])
            nc.scalar.dma_start(out=t_pad, in_=pad[:])
            nc.scalar.dma_start(out=t_causal, in_=causal[:])
            # broadcast the v bias across partitions once (free dim = o)
            t_bv1 = pp.tile([1, OC], F32)
            nc.scalar.dma_start(out=t_bv1, in_=bias_v[:])
            nc.scalar.dma_start(out=t_wo, in_=wo[:].rearrange("(j p) d -> p j d", p=128))
            nc.gpsimd.partition_broadcast(t_bv, t_bv1)
            nc.gpsimd.memset(t_qT[64:128, :, :], 0)
            nc.gpsimd.memset(t_kT[64:128, :, :], 0)
            nc.gpsimd.memset(t_V[:], 0)
            nc.vector.memset(t_V[:, :, :, DK : DK + 1], 1.0)

            # ---- phase A: projections ----
            # q and k land transposed ([o, s], o on partitions); v lands
            # natural ([s, o], s on partitions) for the PV matmul.
            with tc.tile_pool(name="ps_proj", bufs=8, space=PSUM) as ps_proj:
                for name, xin, w_sb, nsc in (("q", xq, t_wq, SC), ("k", xk, t_wk, ksc)):
                    dst = t_qT if name == "q" else t_kT
                    bidx = 0 if name == "q" else 1
                    acc = [
                        ps_proj.tile(
                            [128, 512], F32, tag="proj", name=f"acc_{name}_{n}"
                        )
                        for n in range(OT * nsc)
                    ]
                    xts = []
                    for i in range(IT):
                        xt = xs.tile(
                            [128, nsc * 512], FP16, tag=f"x{name}",
                            name=f"xt_{name}_{i}", bufs=IT,
                        )
                        nc.sync.dma_start(
                            out=xt,
                            in_=xin[i * 128 : (i + 1) * 128, 0 : nsc * 512],
                        )
                        xts.append(xt)
                    for sc in range(nsc):
                        for ot in range(OT):
                            for i in range(IT):
                                nc.tensor.matmul(
                                    acc[ot * nsc + sc],
                                    w_sb[:, i, ot * 128 : (ot + 1) * 128],
                                    xts[i][:, sc * 512 : (sc + 1) * 512],
                                    start=(i == 0),
                                    stop=(i == IT - 1),
                                )
                    for ot in range(OT):
                        for sc in range(nsc):
                            for half in range(2):
                                h = 2 * ot + half
                                p0 = half * 64
                                nc.vector.tensor_scalar_add(
                                    out=dst[0:64, h, sc * 512 : (sc + 1) * 512],
                                    in0=acc[ot * nsc + sc][p0 : p0 + 64, :],
                                    scalar1=t_bqk[p0 : p0 + 64, bidx, ot : ot + 1],
                                )

                # v natural: lhsT = x tile (stationary), rhs = wv (moving).
                # One accumulation group per psum bank (interleaving two
                # start/accumulate groups in one bank corrupts has_written).
                for rnd, sts in enumerate(vrounds):
                    sts = list(sts)
                    w = len(sts) * 128
                    vacc = [
                        ps_proj.tile([128, OC], F32, tag="proj", name=f"vacc_{rnd}_{n}")
                        for n in range(len(sts))
                    ]
                    for i in range(IT):
                        xt = xs.tile([128, w], FP16, tag="xv", name=f"xtv_{rnd}_{i}", bufs=3)
                        nc.sync.dma_start(
                            out=xt,
                            in_=xv[
                                i * 128 : (i + 1) * 128,
                                sts[0] * 128 : sts[0] * 128 + w,
                            ],
                        )
                        for n in range(len(sts)):
                            nc.tensor.matmul(
                                vacc[n],
                                xt[:, n * 128 : (n + 1) * 128],
                                t_wv[:, i, :],
                                start=(i == 0),
                                stop=(i == IT - 1),
                            )
                    for n, st in enumerate(sts):
                        nc.vector.tensor_add(
                            out=t_V[:, st, :, 0:DK],
                            in0=vacc[n].rearrange("p (h d) -> p h d", h=HPG),
                            in1=t_bv.rearrange("p (h d) -> p h d", h=HPG),
                        )

            # ---- phase B: attention (S.T layout) + interleaved phase C ----
            with (
                tc.tile_pool(name="ps_att", bufs=3, space=PSUM) as ps_att,
                tc.tile_pool(name="ps_o", bufs=3, space=PSUM) as ps_o,
                tc.tile_pool(name="ps_c", bufs=2, space=PSUM) as ps_c,
                tc.tile_pool(name="pb", bufs=4) as pb,
                tc.tile_pool(name="nrm", bufs=2) as nrm,
                tc.tile_pool(name="stg", bufs=4) as stg,
            ):
                for qc in range(SC):
                    q0 = qc * 512
                    nkb = min(4 * (qc + 1), kb_cap)
                    for pair in ((0, 1), (2, 3)):
                        o_ps = {
                            h: ps_o.tile(
                                [128, 512], F32, tag="ops", name=f"ops_{qc}_{h}"
                            )
                            for h in pair
                        }
                        for kb in range(nkb):
                            k0 = kb * 128
                            off = max(0, k0 - q0)
                            st = {}
                            for h in pair:
                                st[h] = ps_att.tile(
                                    [128, 512], F32, tag="st", name=f"st_{qc}_{h}_{kb}"
                                )
                                nc.tensor.matmul(
                                    st[h][:, off:512],
                                    t_kT[:, h, k0 : k0 + 128],
                                    t_qT[:, h, q0 + off : q0 + 512],
                                    start=True,
                                    stop=True,
                                )
                            for h in pair:
                                if k0 >= q0:
                                    nc.vector.tensor_add(
                                        out=st[h][:, off : off + 128],
                                        in0=st[h][:, off : off + 128],
                                        in1=t_causal,
                                    )
                                pt = pb.tile(
                                    [128, 512], FP16, tag="pt", name=f"pt_{qc}_{h}_{kb}"
                                )
                                nc.scalar.activation(
                                    out=pt[:, off:512],
                                    in_=st[h][:, off:512],
                                    func=Exp,
                                    bias=t_pad[:, kb : kb + 1],
                                    scale=1.0,
                                )
                                nc.tensor.matmul(
                                    o_ps[h][:, off:512],
                                    t_V[:, kb, h, :],
                                    pt[:, off:512],
                                    start=(kb == 0),
                                    stop=(kb == nkb - 1),
                                )
                        for h in pair:
                            ot, p0 = h // 2, (h % 2) * 64
                            t_l = nrm.tile([128, 512], F32, tag="l", name=f"l_{qc}_{h}")
                            nc.vector.tensor_copy(
                                t_l[0:1, :], o_ps[h][DK : DK + 1, :]
                            )
                            t_r = nrm.tile([128, 512], F32, tag="r", name=f"r_{qc}_{h}")
                            nc.vector.reciprocal_approx_fast(t_r[0:1, :], t_l[0:1, :])
                            t_rb = nrm.tile([DK, 512], F32, tag="rb", name=f"rb_{qc}_{h}")
                            nc.gpsimd.partition_broadcast(t_rb, t_r[0:1, :])
                            nc.vector.tensor_mul(
                                t_OT[p0 : p0 + DK, ot, q0 : q0 + 512],
                                o_ps[h][0:DK, :],
                                t_rb,
                            )
                    # phase C for this 512-chunk of s (needs all 4 heads)
                    for dt_ in range(D // 128):
                        ops = ps_c.tile([128, 512], F32, tag="c", name=f"c_{qc}_{dt_}")
                        for j in range(OT):
                            nc.tensor.matmul(
                                ops,
                                t_wo[:, j, dt_ * 128 : (dt_ + 1) * 128],
                                t_OT[:, j, q0 : q0 + 512],
                                start=(j == 0),
                                stop=(j == OT - 1),
                            )
                        st_o = stg.tile([128, 512], F32, tag="s", name=f"so_{qc}_{dt_}")
                        nc.vector.tensor_copy(st_o, ops)
                        nc.sync.dma_start(
                            out=out_t[dt_ * 128 : (dt_ + 1) * 128, q0 : q0 + 512],
                            in_=st_o,
                        )
    nc.compile()
    return nc


def _get_nc(kb_cap):
    key = ("nc", kb_cap)
    if key not in _cache:
        _cache[key] = _build_nc(kb_cap)
    return _cache[key]


def kernel(
    query,
    key,
    value,
    Wq,
    bq,
    Wk,
    bk,
    Wv,
    bv,
    Wo,
    bo,
    attn_mask,
    key_padding_mask,
):
    import ml_dtypes
    from concourse import bass_utils

    query = np.asarray(query, dtype=np.float32)
    key = np.asarray(key, dtype=np.float32)
    value = np.asarray(value, dtype=np.float32)
    Wq = np.asarray(Wq, dtype=np.float32)
    bq = np.asarray(bq, dtype=np.float32)
    Wk = np.asarray(Wk, dtype=np.float32)
    bk = np.asarray(bk, dtype=np.float32)
    Wv = np.asarray(Wv, dtype=np.float32)
    bv = np.asarray(bv, dtype=np.float32)
    Wo = np.asarray(Wo, dtype=np.float32)
    bo = np.asarray(bo, dtype=np.float32)
    attn_mask = np.asarray(attn_mask)
    key_padding_mask = np.asarray(key_padding_mask)

    # this kernel hardcodes the causal structure of attn_mask
    expected = np.triu(np.ones((S, S), dtype=bool), k=1)
    assert np.array_equal(attn_mask, expected), "kernel assumes causal attn_mask"

    # number of 128-blocks that contain any valid (unpadded) key
    valid = ~key_padding_mask  # [B, S]
    kb_cap = 0
    for b in range(B):
        nz = np.nonzero(valid[b])[0]
        cap = (int(nz.max()) // 128 + 1) if nz.size else 1
        kb_cap = max(kb_cap, cap)

    scale = np.float32(1.0 / np.sqrt(DK))
    causal_tile = np.where(
        np.arange(128)[None, :] >= np.arange(128)[:, None], 0.0, -60000.0
    ).astype(np.float16)

    # per-batch transposed activations (shared by the batch's 4 cores)
    xq_b = [np.ascontiguousarray(query[:, b, :].T.astype(np.float16)) for b in range(B)]
    xk_b = [np.ascontiguousarray(key[:, b, :].T.astype(np.float16)) for b in range(B)]
    xv_b = [np.ascontiguousarray(value[:, b, :].T.astype(np.float16)) for b in range(B)]
    pad_b = [
        np.ascontiguousarray(
            np.where(key_padding_mask[b], NEG, 0.0)
            .astype(np.float32)
            .reshape(KB, 128)
            .T
        )
        for b in range(B)
    ]

    in_maps = []
    for c in range(N_CORES):
        b = c // GROUPS
        g = c % GROUPS
        o0 = g * OC
        osl = slice(o0, o0 + OC)
        bias_qk = np.stack(
            [
                (bq[osl] * scale).reshape(OT, 128).T,
                bk[osl].reshape(OT, 128).T,
            ],
            axis=1,
        ).astype(np.float32)  # [128, 2, OT]
        in_maps.append(
            {
                "xq": xq_b[b],
                "xk": xk_b[b],
                "xv": xv_b[b],
                "wq": np.ascontiguousarray((Wq[osl, :] * scale).T.astype(np.float16)),
                "wk": np.ascontiguousarray(Wk[osl, :].T.astype(np.float16)),
                "wv": np.ascontiguousarray(Wv[osl, :].T.astype(np.float16)),
                "wo": np.ascontiguousarray(Wo[:, osl].T).astype(np.float16),
                "bias_qk": np.ascontiguousarray(bias_qk),
                "bias_v": np.ascontiguousarray(bv[osl][None, :]),
                "pad": pad_b[b],
                "causal": causal_tile,
            }
        )

    res = bass_utils.run_bass_kernel_spmd(
        _get_nc(kb_cap), in_maps, core_ids=list(range(N_CORES))
    )
    _cache["last_res"] = res

    out = np.zeros((S, B, D), dtype=np.float32)
    for b in range(B):
        acc = np.zeros((D, S), dtype=np.float32)
        for g in range(GROUPS):
            acc += res.results[b * GROUPS + g]["out_t"]
        out[:, b, :] = acc.T + bo[None, :]
    return out



# revision 3
# speedup vs baseline: 1.0013x; 1.0013x over previous
"""Trainium2 Bass kernel for nn_MultiHeadAttention_59614146068609.

Sharding: 8 cores = 2 batches x 4 head-groups (4 heads each).
Each core projects q/k/v for its batch with its head-slice of Wq/Wk/Wv
(column-sharded), runs causal+padded attention for its 4 heads, and
applies its row-slice of Wo, producing a partial [D, S] fp16 output.
The host sums the 4 partials per batch and adds bo.

Layout: q/k land transposed and PAIR-PACKED ([128, pair, s] with head
2p in partitions 0:64 and head 2p+1 in 64:128), so projections evict
full-width tiles and attention runs 64-contraction matmuls at partition
offsets 0/64 (PE quadrant tile_position). Scores for a head-pair land
in one 2-bank PSUM tile [128, 2, 512] so a single ACT exp instruction
covers both heads (the scalar engine is the scarce resource). V is
natural layout with an appended ones-column providing softmax sums.

Schedule: x DMA is prefetched per 512-chunk; q/k projection for chunk
qc and v projection for the next qc's key blocks are interleaved into
the attention loop so the tensor engine streams continuously (TRN2 PE
p-state ramps to 2.4 GHz only under continuous execution). Attention
processes both head-pairs interleaved per key-block with PV trailing
scores by one block to hide exp latency.

Specialized at build time on kb_cap = number of 128-wide key blocks
that contain any unpadded key; fully padded key blocks are skipped.
"""

import numpy as np

S = 2048
B = 2
D = 1024
H = 16
DK = 64
N_CORES = 8
GROUPS = N_CORES // B          # head groups per batch = 4
HPG = H // GROUPS              # heads per group = 4
OC = HPG * DK                  # per-core projected dim = 256
OT = OC // 128                 # o-tiles / head-pairs per core = 2
IT = D // 128                  # contraction tiles = 8
SC = S // 512                  # sequence chunks of 512 = 4
KB = S // 128                  # k blocks of 128 = 16
NEG = -1e30

_cache = {}


def _build_nc(kb_cap):
    import concourse.bacc as bacc
    import concourse.bass as bass
    import concourse.mybir as mybir
    import concourse.tile as tile
    from concourse import library_config

    F32 = mybir.dt.float32
    FP16 = mybir.dt.float16
    Exp = mybir.ActivationFunctionType.Exp
    PSUM = bass.MemorySpace.PSUM

    ksc = -(-kb_cap * 128 // 512)        # 512-chunks of k to project
    nkb = [min(4 * (qc + 1), kb_cap) for qc in range(SC)]

    def vblocks(qc):
        # v key-blocks first needed by attention chunk qc
        lo = 4 * qc
        hi = min(4 * (qc + 1), kb_cap)
        return list(range(lo, hi))

    nc = bacc.Bacc("TRN2", target_bir_lowering=False, debug=False)

    xq = nc.dram_tensor("xq", [D, S], FP16, kind="ExternalInput")
    xk = nc.dram_tensor("xk", [D, S], FP16, kind="ExternalInput")
    xv = nc.dram_tensor("xv", [D, S], FP16, kind="ExternalInput")
    wq = nc.dram_tensor("wq", [D, OC], FP16, kind="ExternalInput")
    wk = nc.dram_tensor("wk", [D, OC], FP16, kind="ExternalInput")
    wv = nc.dram_tensor("wv", [D, OC], FP16, kind="ExternalInput")
    wo = nc.dram_tensor("wo", [OC, D], FP16, kind="ExternalInput")
    bias_qk = nc.dram_tensor("bias_qk", [128, 2, OT], F32, kind="ExternalInput")
    bias_v = nc.dram_tensor("bias_v", [1, OC], F32, kind="ExternalInput")
    pad = nc.dram_tensor("pad", [128, KB], F32, kind="ExternalInput")
    causal2 = nc.dram_tensor("causal2", [128, 2, 128], FP16, kind="ExternalInput")
    out_t = nc.dram_tensor("out_t", [D, S], FP16, kind="ExternalOutput")

    with tile.TileContext(nc) as tc, nc.allow_low_precision(
        reason="fp16 compute throughout; validated vs fp32 reference"
    ):
        with (
            tc.tile_pool(name="persist", bufs=1) as pp,
            tc.tile_pool(name="xs", bufs=16) as xs,
            tc.tile_pool(name="ptp", bufs=6) as ptp,
            tc.tile_pool(name="nrm", bufs=2) as nrm,
            tc.tile_pool(name="stg", bufs=4) as stg,
            tc.tile_pool(name="ps", bufs=2, space=PSUM) as ps,
        ):
            nc.gpsimd.load_library(library_config.attn)

            # ---- persistent SBUF tensors ----
            t_wq = pp.tile([128, IT, OC], FP16)
            t_wk = pp.tile([128, IT, OC], FP16)
            t_wv = pp.tile([128, IT, OC], FP16)
            t_wo = pp.tile([128, OT, D], FP16)
            t_bqk = pp.tile([128, 2, OT], F32)
            t_bv = pp.tile([128, OC], F32)
            t_bv1 = pp.tile([1, OC], F32)
            t_pad = pp.tile([128, KB], F32)
            t_causal2 = pp.tile([128, 2, 128], FP16)
            t_qT = pp.tile([128, OT, S], FP16)
            t_kT = pp.tile([128, OT, ksc * 512], FP16)
            t_V = pp.tile([128, kb_cap, HPG, DK + 1], FP16)
            t_OT = pp.tile([128, OT, S], FP16)

            # weight/constant DMAs on the scalar queue (ACT is idle early)
            nc.scalar.dma_start(out=t_wq, in_=wq[:].rearrange("(i p) o -> p i o", p=128))
            nc.scalar.dma_start(out=t_wk, in_=wk[:].rearrange("(i p) o -> p i o", p=128))
            nc.scalar.dma_start(out=t_wv, in_=wv[:].rearrange("(i p) o -> p i o", p=128))
            nc.scalar.dma_start(out=t_wo, in_=wo[:].rearrange("(j p) d -> p j d", p=128))
            nc.scalar.dma_start(out=t_bqk, in_=bias_qk[:])
            nc.scalar.dma_start(out=t_bv1, in_=bias_v[:])
            nc.scalar.dma_start(out=t_pad, in_=pad[:])
            nc.scalar.dma_start(out=t_causal2, in_=causal2[:])
            nc.gpsimd.partition_broadcast(t_bv, t_bv1)
            # softmax-denominator ones column of V
            nc.vector.memset(t_V[:, :, :, DK : DK + 1], 1.0)

            # ---- x tile DMA helpers (sync queue) ----
            xq_t = {}
            xk_t = {}
            xv_t = {}

            def dma_xchunk(sc):
                if sc < SC:
                    for i in range(IT):
                        t = xs.tile([128, 512], FP16, tag="xq", name=f"xq_{sc}_{i}")
                        nc.sync.dma_start(
                            out=t,
                            in_=xq[i * 128 : (i + 1) * 128, sc * 512 : (sc + 1) * 512],
                        )
                        xq_t[(sc, i)] = t
                if sc < ksc:
                    for i in range(IT):
                        t = xs.tile([128, 512], FP16, tag="xk", name=f"xk_{sc}_{i}")
                        nc.sync.dma_start(
                            out=t,
                            in_=xk[i * 128 : (i + 1) * 128, sc * 512 : (sc + 1) * 512],
                        )
                        xk_t[(sc, i)] = t

            def dma_xv(blocks):
                if not blocks:
                    return
                w = len(blocks) * 128
                c0 = blocks[0] * 128
                for i in range(IT):
                    t = xs.tile([128, 512], FP16, tag="xv", name=f"xv_{blocks[0]}_{i}")
                    nc.sync.dma_start(
                        out=t[:, 0:w], in_=xv[i * 128 : (i + 1) * 128, c0 : c0 + w]
                    )
                    xv_t[(blocks[0], i)] = t

            # ---- projection helpers ----
            def proj_qk(sc, w_sb, bidx, dst, xtiles):
                for ot in range(OT):
                    acc = ps.tile(
                        [128, 2, 512], F32, tag="w", bufs=2, name=f"a{bidx}_{sc}_{ot}"
                    )
                    for i in range(IT):
                        nc.tensor.matmul(
                            acc[:, 0, :],
                            w_sb[:, i, ot * 128 : (ot + 1) * 128],
                            xtiles[i],
                            start=(i == 0),
                            stop=(i == IT - 1),
                        )
                    nc.vector.tensor_scalar_add(
                        out=dst[:, ot, sc * 512 : (sc + 1) * 512],
                        in0=acc[:, 0, :],
                        scalar1=t_bqk[:, bidx, ot : ot + 1],
                    )

            def proj_v(blocks):
                if not blocks:
                    return
                b0 = blocks[0]
                for n, blk in enumerate(blocks):
                    vacc = ps.tile(
                        [128, 2, 512], F32, tag="w", bufs=2, name=f"v_{blk}"
                    )
                    for i in range(IT):
                        nc.tensor.matmul(
                            vacc[:, 0, 0:OC],
                            xv_t[(b0, i)][:, n * 128 : (n + 1) * 128],
                            t_wv[:, i, :],
                            start=(i == 0),
                            stop=(i == IT - 1),
                        )
                    nc.vector.tensor_add(
                        out=t_V[:, blk, :, 0:DK],
                        in0=vacc[:, 0, 0:OC].rearrange("p (h d) -> p h d", h=HPG),
                        in1=t_bv.rearrange("p (h d) -> p h d", h=HPG),
                    )

            # ---- fused main loop ----
            dma_xchunk(0)
            dma_xv(vblocks(0))

            for qc in range(SC):
                q0 = qc * 512
                # prefetch next chunk's inputs
                dma_xchunk(qc + 1)
                dma_xv(vblocks(qc + 1) if qc + 1 < SC else [])

                # project this chunk's q (and k while chunks remain)
                proj_qk(qc, t_wq, 0, t_qT, [xq_t[(qc, i)] for i in range(IT)])
                if qc < ksc:
                    proj_qk(qc, t_wk, 1, t_kT, [xk_t[(qc, i)] for i in range(IT)])
                if qc == 0:
                    proj_v(vblocks(0))

                # attention: both pairs interleaved, PV trails scores by 1 blk
                o_ps = {
                    (p, hh): ps.tile(
                        [128, 512], F32, tag="o", bufs=4, name=f"o_{qc}_{p}_{hh}"
                    )
                    for p in range(OT)
                    for hh in range(2)
                }
                last = nkb[qc] - 1

                def emit_pv(kb, off, pts):
                    for p in range(OT):
                        for hh in range(2):
                            nc.tensor.matmul(
                                o_ps[(p, hh)][0 : DK + 1, off:512],
                                t_V[:, kb, 2 * p + hh, :],
                                pts[p][:, hh, off:512],
                                start=(kb == 0),
                                stop=(kb == last),
                            )

                prev = None
                for kb in range(nkb[qc]):
                    k0 = kb * 128
                    off = max(0, k0 - q0)
                    st2s = {}
                    for p in range(OT):
                        st2 = ps.tile(
                            [128, 2, 512], F32, tag="w", bufs=2,
                            name=f"st_{qc}_{kb}_{p}",
                        )
                        for hh in range(2):
                            nc.tensor.matmul(
                                st2[:, hh, off:512],
                                t_kT[hh * 64 : (hh + 1) * 64, p, k0 : k0 + 128],
                                t_qT[hh * 64 : (hh + 1) * 64, p, q0 + off : q0 + 512],
                                start=True,
                                stop=True,
                            )
                        st2s[p] = st2
                    if prev is not None:
                        emit_pv(*prev)
                    pts = {}
                    for p in range(OT):
                        if k0 >= q0:
                            nc.vector.tensor_add(
                                out=st2s[p][:, :, off : off + 128],
                                in0=st2s[p][:, :, off : off + 128],
                                in1=t_causal2,
                            )
                        pt = ptp.tile(
                            [128, 2, 512], FP16, tag="pt", name=f"pt_{qc}_{kb}_{p}"
                        )
                        nc.scalar.activation(
                            out=pt[:, :, off:512],
                            in_=st2s[p][:, :, off:512],
                            func=Exp,
                            bias=t_pad[:, kb : kb + 1],
                            scale=1.0,
                        )
                        pts[p] = pt
                    prev = (kb, off, pts)
                emit_pv(*prev)

                # normalize by the ones-column sums -> t_OT
                for p in range(OT):
                    for hh in range(2):
                        t_l = nrm.tile([1, 512], F32, tag="l", name=f"l_{qc}_{p}_{hh}")
                        nc.vector.tensor_copy(t_l[0:1, :], o_ps[(p, hh)][DK : DK + 1, :])
                        t_r = nrm.tile([1, 512], F32, tag="r", name=f"r_{qc}_{p}_{hh}")
                        nc.vector.reciprocal_approx_fast(t_r[0:1, :], t_l[0:1, :])
                        t_rb = nrm.tile(
                            [DK, 512], F32, tag="rb", name=f"rb_{qc}_{p}_{hh}"
                        )
                        nc.gpsimd.partition_broadcast(t_rb, t_r[0:1, :])
                        nc.vector.tensor_mul(
                            t_OT[hh * 64 : (hh + 1) * 64, p, q0 : q0 + 512],
                            o_ps[(p, hh)][0:DK, :],
                            t_rb,
                        )

                # v projection for the next chunk's key blocks (PE filler)
                if qc + 1 < SC:
                    proj_v(vblocks(qc + 1))

                # output projection for this chunk
                for dt_ in range(D // 128):
                    if dt_ % 2 == 0:
                        ops = ps.tile(
                            [128, 2, 512], F32, tag="w", bufs=2, name=f"c_{qc}_{dt_}"
                        )
                        opsv = ops[:, 0, :]
                    else:
                        ops = ps.tile(
                            [128, 512], F32, tag="o", bufs=4, name=f"c_{qc}_{dt_}"
                        )
                        opsv = ops[:, :]
                    for j in range(OT):
                        nc.tensor.matmul(
                            opsv,
                            t_wo[:, j, dt_ * 128 : (dt_ + 1) * 128],
                            t_OT[:, j, q0 : q0 + 512],
                            start=(j == 0),
                            stop=(j == OT - 1),
                        )
                    st_o = stg.tile([128, 512], FP16, tag="s", name=f"so_{qc}_{dt_}")
                    nc.vector.tensor_copy(st_o, opsv)
                    nc.gpsimd.dma_start(
                        out=out_t[dt_ * 128 : (dt_ + 1) * 128, q0 : q0 + 512],
                        in_=st_o,
                    )
    nc.compile()
    return nc


def _get_nc(kb_cap):
    key = ("nc", kb_cap)
    if key not in _cache:
        _cache[key] = _build_nc(kb_cap)
    return _cache[key]


def kernel(
    query,
    key,
    value,
    Wq,
    bq,
    Wk,
    bk,
    Wv,
    bv,
    Wo,
    bo,
    attn_mask,
    key_padding_mask,
):
    from concourse import bass_utils

    query = np.asarray(query, dtype=np.float32)
    key = np.asarray(key, dtype=np.float32)
    value = np.asarray(value, dtype=np.float32)
    Wq = np.asarray(Wq, dtype=np.float32)
    bq = np.asarray(bq, dtype=np.float32)
    Wk = np.asarray(Wk, dtype=np.float32)
    bk = np.asarray(bk, dtype=np.float32)
    Wv = np.asarray(Wv, dtype=np.float32)
    bv = np.asarray(bv, dtype=np.float32)
    Wo = np.asarray(Wo, dtype=np.float32)
    bo = np.asarray(bo, dtype=np.float32)
    attn_mask = np.asarray(attn_mask)
    key_padding_mask = np.asarray(key_padding_mask)

    # this kernel hardcodes the causal structure of attn_mask
    expected = np.triu(np.ones((S, S), dtype=bool), k=1)
    assert np.array_equal(attn_mask, expected), "kernel assumes causal attn_mask"

    # number of 128-blocks that contain any valid (unpadded) key
    valid = ~key_padding_mask  # [B, S]
    kb_cap = 0
    for b in range(B):
        nz = np.nonzero(valid[b])[0]
        cap = (int(nz.max()) // 128 + 1) if nz.size else 1
        kb_cap = max(kb_cap, cap)

    scale = np.float32(1.0 / np.sqrt(DK))
    causal_tile = np.where(
        np.arange(128)[None, :] >= np.arange(128)[:, None], 0.0, -60000.0
    ).astype(np.float16)
    causal2 = np.ascontiguousarray(
        np.stack([causal_tile, causal_tile], axis=1)
    )  # [128, 2, 128]

    # per-batch transposed activations (shared by the batch's 4 cores)
    xq_b = [np.ascontiguousarray(query[:, b, :].T.astype(np.float16)) for b in range(B)]
    xk_b = [np.ascontiguousarray(key[:, b, :].T.astype(np.float16)) for b in range(B)]
    xv_b = [np.ascontiguousarray(value[:, b, :].T.astype(np.float16)) for b in range(B)]
    pad_b = [
        np.ascontiguousarray(
            np.where(key_padding_mask[b], NEG, 0.0)
            .astype(np.float32)
            .reshape(KB, 128)
            .T
        )
        for b in range(B)
    ]

    in_maps = []
    for c in range(N_CORES):
        b = c // GROUPS
        g = c % GROUPS
        o0 = g * OC
        osl = slice(o0, o0 + OC)
        bias_qk = np.stack(
            [
                (bq[osl] * scale).reshape(OT, 128).T,
                bk[osl].reshape(OT, 128).T,
            ],
            axis=1,
        ).astype(np.float32)  # [128, 2, OT]
        in_maps.append(
            {
                "xq": xq_b[b],
                "xk": xk_b[b],
                "xv": xv_b[b],
                "wq": np.ascontiguousarray((Wq[osl, :] * scale).T.astype(np.float16)),
                "wk": np.ascontiguousarray(Wk[osl, :].T.astype(np.float16)),
                "wv": np.ascontiguousarray(Wv[osl, :].T.astype(np.float16)),
                "wo": np.ascontiguousarray(Wo[:, osl].T).astype(np.float16),
                "bias_qk": np.ascontiguousarray(bias_qk),
                "bias_v": np.ascontiguousarray(bv[osl][None, :]),
                "pad": pad_b[b],
                "causal2": causal2,
            }
        )

    res = bass_utils.run_bass_kernel_spmd(
        _get_nc(kb_cap), in_maps, core_ids=list(range(N_CORES))
    )
    _cache["last_res"] = res

    out = np.zeros((S, B, D), dtype=np.float32)
    for b in range(B):
        acc = np.zeros((D, S), dtype=np.float32)
        for g in range(GROUPS):
            acc += res.results[b * GROUPS + g]["out_t"].astype(np.float32)
        out[:, b, :] = acc.T + bo[None, :]
    return out


# revision 8
# speedup vs baseline: 1.0409x; 1.0395x over previous
"""Trainium2 Bass kernel for nn_MultiHeadAttention_59614146068609.

Sharding: 8 cores = 2 batches x 4 head-groups (4 heads each).
Each core projects q/k/v for its batch with its head-slice of Wq/Wk/Wv
(column-sharded), runs causal+padded attention for its 4 heads, and
applies its row-slice of Wo, producing a partial [D, S] fp16 output.
The host sums the 4 partials per batch and adds bo.

Layout: q/k land transposed and PAIR-PACKED ([128, pair, s] with head
2p in partitions 0:64 and head 2p+1 in 64:128), so projections evict
full-width tiles and attention runs 64-contraction matmuls at partition
offsets 0/64 (PE quadrant tile_position). Scores for a head-pair land
in one 2-bank PSUM tile [128, 2, 512] so a single ACT exp instruction
covers both heads (the scalar engine is the scarce resource). V is
natural layout with an appended ones-column providing softmax sums.

Schedule: the kernel is one software pipeline. Attention for chunk qc
interleaves, per key-block step, "filler" tensor work units (q/k/v
projections for qc+1 and the Wo output projection of qc-1) popped from
a queue, so the tensor engine always has independent work while the
scalar engine's exp chain catches up (TRN2 PE p-state ramps to 2.4 GHz
only under continuous execution). PV trails scores by one key block.

Specialized at build time on kb_cap = number of 128-wide key blocks
that contain any unpadded key; fully padded key blocks are skipped.
"""

from collections import deque

import numpy as np

S = 2048
B = 2
D = 1024
H = 16
DK = 64
N_CORES = 8
GROUPS = N_CORES // B          # head groups per batch = 4
HPG = H // GROUPS              # heads per group = 4
OC = HPG * DK                  # per-core projected dim = 256
OT = OC // 128                 # o-tiles / head-pairs per core = 2
IT = D // 128                  # contraction tiles = 8
SC = S // 512                  # sequence chunks of 512 = 4
KB = S // 128                  # k blocks of 128 = 16
NEG = -1e30

_cache = {}


def _build_nc(kb_cap):
    import concourse.bacc as bacc
    import concourse.bass as bass
    import concourse.mybir as mybir
    import concourse.tile as tile
    from concourse import library_config

    F32 = mybir.dt.float32
    FP16 = mybir.dt.float16
    Exp = mybir.ActivationFunctionType.Exp
    PSUM = bass.MemorySpace.PSUM

    ksc = -(-kb_cap * 128 // 512)        # 512-chunks of k to project
    nkb = [min(4 * (qc + 1), kb_cap) for qc in range(SC)]

    def vblocks(qc):
        # v key-blocks first needed by attention chunk qc
        if qc >= SC:
            return []
        return list(range(4 * qc, min(4 * (qc + 1), kb_cap)))

    nc = bacc.Bacc("TRN2", target_bir_lowering=False, debug=False)

    xq = nc.dram_tensor("xq", [D, S], FP16, kind="ExternalInput")
    xk = nc.dram_tensor("xk", [D, S], FP16, kind="ExternalInput")
    xv = nc.dram_tensor("xv", [D, S], FP16, kind="ExternalInput")
    wq = nc.dram_tensor("wq", [D, OC], FP16, kind="ExternalInput")
    wk = nc.dram_tensor("wk", [D, OC], FP16, kind="ExternalInput")
    wv = nc.dram_tensor("wv", [D, OC], FP16, kind="ExternalInput")
    wo = nc.dram_tensor("wo", [OC, D], FP16, kind="ExternalInput")
    bias_qk = nc.dram_tensor("bias_qk", [128, 2, OT], F32, kind="ExternalInput")
    bias_v = nc.dram_tensor("bias_v", [1, OC], F32, kind="ExternalInput")
    pad = nc.dram_tensor("pad", [128, KB], F32, kind="ExternalInput")
    causal2 = nc.dram_tensor("causal2", [128, 2, 128], FP16, kind="ExternalInput")
    out_t = nc.dram_tensor("out_t", [D, S], FP16, kind="ExternalOutput")

    with tile.TileContext(nc) as tc, nc.allow_low_precision(
        reason="fp16 compute throughout; validated vs fp32 reference"
    ):
        with (
            tc.tile_pool(name="persist", bufs=1) as pp,
            tc.tile_pool(name="xs", bufs=2) as xs,
            tc.tile_pool(name="ptp", bufs=6) as ptp,
            tc.tile_pool(name="nrm", bufs=2) as nrm,
            tc.tile_pool(name="stg", bufs=4) as stg,
            tc.tile_pool(name="ps", bufs=2, space=PSUM) as ps,
        ):
            nc.gpsimd.load_library(library_config.attn)

            # ---- persistent SBUF tensors ----
            t_wq = pp.tile([128, IT, OC], FP16)
            t_wk = pp.tile([128, IT, OC], FP16)
            t_wv = pp.tile([128, IT, OC], FP16)
            t_wo = pp.tile([128, OT, D], FP16)
            t_bqk = pp.tile([128, 2, OT], F32)
            t_bv = pp.tile([128, OC], F32)
            t_bv1 = pp.tile([1, OC], F32)
            t_pad = pp.tile([128, KB], F32)
            t_causal2 = pp.tile([128, 2, 128], FP16)
            t_qT = pp.tile([128, OT, S], FP16)
            t_kT = pp.tile([128, OT, ksc * 512], FP16)
            t_V = pp.tile([128, kb_cap, HPG, DK + 1], FP16)
            t_OT = pp.tile([128, OT, S], FP16)

            # weight/constant DMAs on the scalar queue (ACT is idle early)
            nc.scalar.dma_start(out=t_wq, in_=wq[:].rearrange("(i p) o -> p i o", p=128))
            nc.scalar.dma_start(out=t_wk, in_=wk[:].rearrange("(i p) o -> p i o", p=128))
            nc.scalar.dma_start(out=t_bqk, in_=bias_qk[:])
            nc.scalar.dma_start(out=t_wv, in_=wv[:].rearrange("(i p) o -> p i o", p=128))
            nc.scalar.dma_start(out=t_bv1, in_=bias_v[:])
            nc.scalar.dma_start(out=t_pad, in_=pad[:])
            nc.scalar.dma_start(out=t_causal2, in_=causal2[:])
            nc.scalar.dma_start(out=t_wo, in_=wo[:].rearrange("(j p) d -> p j d", p=128))
            nc.gpsimd.partition_broadcast(t_bv, t_bv1)
            # softmax-denominator ones column of V
            nc.vector.memset(t_V[:, :, :, DK : DK + 1], 1.0)

            # ---- chunk-granular x DMAs (sync queue) ----
            xq_t = {}
            xk_t = {}
            xv_t = {}

            def dma_xchunk(sc):
                if sc < SC:
                    t = xs.tile([128, IT, 512], FP16, tag="xq", name=f"xq_{sc}")
                    nc.sync.dma_start(
                        out=t,
                        in_=xq[:, sc * 512 : (sc + 1) * 512].rearrange(
                            "(i p) s -> p i s", p=128
                        ),
                    )
                    xq_t[sc] = t
                if sc < ksc:
                    t = xs.tile([128, IT, 512], FP16, tag="xk", name=f"xk_{sc}")
                    nc.sync.dma_start(
                        out=t,
                        in_=xk[:, sc * 512 : (sc + 1) * 512].rearrange(
                            "(i p) s -> p i s", p=128
                        ),
                    )
                    xk_t[sc] = t

            def dma_xv(blocks):
                if not blocks:
                    return
                w = len(blocks) * 128
                c0 = blocks[0] * 128
                t = xs.tile([128, IT, 512], FP16, tag="xv", name=f"xv_{blocks[0]}")
                nc.sync.dma_start(
                    out=t[:, :, 0:w],
                    in_=xv[:, c0 : c0 + w].rearrange("(i p) s -> p i s", p=128),
                )
                xv_t[blocks[0]] = t

            # ---- work units (each emits one PE matmul group + eviction) ----
            def unit_qk(sc, w_sb, bidx, dst, xt, ot):
                def emit():
                    acc = ps.tile(
                        [128, 2, 512], F32, tag="w", bufs=2, name=f"a{bidx}_{sc}_{ot}"
                    )
                    for i in range(IT):
                        nc.tensor.matmul(
                            acc[:, 0, :],
                            w_sb[:, i, ot * 128 : (ot + 1) * 128],
                            xt[:, i, :],
                            start=(i == 0),
                            stop=(i == IT - 1),
                        )
                    nc.vector.tensor_scalar_add(
                        out=dst[:, ot, sc * 512 : (sc + 1) * 512],
                        in0=acc[:, 0, :],
                        scalar1=t_bqk[:, bidx, ot : ot + 1],
                    )
                return emit

            def unit_v(b0, n, blk):
                def emit():
                    xt = xv_t[b0]
                    vacc = ps.tile([128, 2, 512], F32, tag="w", bufs=2, name=f"v_{blk}")
                    for i in range(IT):
                        nc.tensor.matmul(
                            vacc[:, 0, 0:OC],
                            xt[:, i, n * 128 : (n + 1) * 128],
                            t_wv[:, i, :],
                            start=(i == 0),
                            stop=(i == IT - 1),
                        )
                    nc.vector.tensor_add(
                        out=t_V[:, blk, :, 0:DK],
                        in0=vacc[:, 0, 0:OC].rearrange("p (h d) -> p h d", h=HPG),
                        in1=t_bv.rearrange("p (h d) -> p h d", h=HPG),
                    )
                return emit

            def unit_phase_c(qc, dt_):
                def emit():
                    q0 = qc * 512
                    ops = ps.tile(
                        [128, 2, 512], F32, tag="w", bufs=2, name=f"c_{qc}_{dt_}"
                    )
                    opsv = ops[:, 0, :]
                    for j in range(OT):
                        nc.tensor.matmul(
                            opsv,
                            t_wo[:, j, dt_ * 128 : (dt_ + 1) * 128],
                            t_OT[:, j, q0 : q0 + 512],
                            start=(j == 0),
                            stop=(j == OT - 1),
                        )
                    st_o = stg.tile([128, 512], FP16, tag="s", name=f"so_{qc}_{dt_}")
                    nc.vector.tensor_copy(st_o, opsv)
                    nc.gpsimd.dma_start(
                        out=out_t[dt_ * 128 : (dt_ + 1) * 128, q0 : q0 + 512],
                        in_=st_o,
                    )
                return emit

            def proj_units(sc):
                u = []
                if sc < SC:
                    for ot in range(OT):
                        u.append(unit_qk(sc, t_wq, 0, t_qT, xq_t[sc], ot))
                if sc < ksc:
                    for ot in range(OT):
                        u.append(unit_qk(sc, t_wk, 1, t_kT, xk_t[sc], ot))
                vb = vblocks(sc)
                if vb:
                    for n, blk in enumerate(vb):
                        u.append(unit_v(vb[0], n, blk))
                return u

            # ---- fused pipelined main loop ----
            dma_xchunk(0)
            dma_xv(vblocks(0))
            dma_xchunk(1)
            dma_xv(vblocks(1))

            fillers = deque()
            # chunk 0's q/k projections must precede its first scores matmul;
            # its v projections interleave into the loop (PV trails 1 block)
            fillers.extend(proj_units(0))
            n_fill0 = 2 * OT if ksc > 0 else OT

            for qc in range(SC):
                q0 = qc * 512
                # prefetch chunk qc+2 inputs; queue chunk qc+1 projections
                dma_xchunk(qc + 2)
                dma_xv(vblocks(qc + 2))
                if qc + 1 < SC:
                    fillers.extend(proj_units(qc + 1))

                o_ps = {
                    (p, hh): ps.tile(
                        [128, 512], F32, tag="o", bufs=4, name=f"o_{qc}_{p}_{hh}"
                    )
                    for p in range(OT)
                    for hh in range(2)
                }
                last = nkb[qc] - 1

                def emit_pv(kb, off, pts, o_ps=o_ps, last=last):
                    for p in range(OT):
                        for hh in range(2):
                            nc.tensor.matmul(
                                o_ps[(p, hh)][0 : DK + 1, off:512],
                                t_V[:, kb, 2 * p + hh, :],
                                pts[p][:, hh, off:512],
                                start=(kb == 0),
                                stop=(kb == last),
                            )

                # for chunk 0, the first steps must emit this chunk's own
                # q/k/v projections before any attention matmul can run
                if qc == 0:
                    for _ in range(n_fill0):
                        fillers.popleft()()

                steps = nkb[qc]
                prev = None
                for kb in range(steps):
                    k0 = kb * 128
                    off = max(0, k0 - q0)
                    st2s = {}
                    for p in range(OT):
                        st2 = ps.tile(
                            [128, 2, 512], F32, tag="w", bufs=2,
                            name=f"st_{qc}_{kb}_{p}",
                        )
                        for hh in range(2):
                            nc.tensor.matmul(
                                st2[:, hh, off:512],
                                t_kT[hh * 64 : (hh + 1) * 64, p, k0 : k0 + 128],
                                t_qT[hh * 64 : (hh + 1) * 64, p, q0 + off : q0 + 512],
                                start=True,
                                stop=True,
                            )
                        st2s[p] = st2
                    if prev is not None:
                        emit_pv(*prev)
                    pts = {}
                    for p in range(OT):
                        if k0 >= q0:
                            nc.vector.tensor_add(
                                out=st2s[p][:, :, off : off + 128],
                                in0=st2s[p][:, :, off : off + 128],
                                in1=t_causal2,
                            )
                        pt = ptp.tile(
                            [128, 2, 512], FP16, tag="pt", name=f"pt_{qc}_{kb}_{p}"
                        )
                        nc.scalar.activation(
                            out=pt[:, :, off:512],
                            in_=st2s[p][:, :, off:512],
                            func=Exp,
                            bias=t_pad[:, kb : kb + 1],
                            scale=1.0,
                        )
                        pts[p] = pt
                    # interleave filler tensor work (emitted AFTER this
                    # block's causal+exp so PSUM slot rotation can never
                    # order a filler's eviction ahead of the exp chain that
                    # releases its slot)
                    remaining = steps - kb
                    n_pop = max(1, -(-len(fillers) // remaining)) if fillers else 0
                    for _ in range(min(n_pop, len(fillers))):
                        fillers.popleft()()
                    prev = (kb, off, pts)
                emit_pv(*prev)

                # normalize by the ones-column sums -> t_OT
                for p in range(OT):
                    for hh in range(2):
                        t_l = nrm.tile([1, 512], F32, tag="l", name=f"l_{qc}_{p}_{hh}")
                        nc.vector.tensor_copy(t_l[0:1, :], o_ps[(p, hh)][DK : DK + 1, :])
                        t_r = nrm.tile([1, 512], F32, tag="r", name=f"r_{qc}_{p}_{hh}")
                        nc.vector.reciprocal_approx_fast(t_r[0:1, :], t_l[0:1, :])
                        t_rb = nrm.tile(
                            [DK, 512], F32, tag="rb", name=f"rb_{qc}_{p}_{hh}"
                        )
                        nc.gpsimd.partition_broadcast(t_rb, t_r[0:1, :])
                        nc.vector.tensor_mul(
                            t_OT[hh * 64 : (hh + 1) * 64, p, q0 : q0 + 512],
                            o_ps[(p, hh)][0:DK, :],
                            t_rb,
                        )

                # output projection of this chunk runs interleaved into the
                # next chunk's attention; the final chunk's runs at the tail
                if qc + 1 < SC:
                    fillers.extend(unit_phase_c(qc, dt_) for dt_ in range(D // 128))
                else:
                    for dt_ in range(D // 128):
                        unit_phase_c(qc, dt_)()
    nc.compile()
    return nc


def _get_nc(kb_cap):
    key = ("nc", kb_cap)
    if key not in _cache:
        _cache[key] = _build_nc(kb_cap)
    return _cache[key]


def kernel(
    query,
    key,
    value,
    Wq,
    bq,
    Wk,
    bk,
    Wv,
    bv,
    Wo,
    bo,
    attn_mask,
    key_padding_mask,
):
    from concourse import bass_utils

    query = np.asarray(query, dtype=np.float32)
    key = np.asarray(key, dtype=np.float32)
    value = np.asarray(value, dtype=np.float32)
    Wq = np.asarray(Wq, dtype=np.float32)
    bq = np.asarray(bq, dtype=np.float32)
    Wk = np.asarray(Wk, dtype=np.float32)
    bk = np.asarray(bk, dtype=np.float32)
    Wv = np.asarray(Wv, dtype=np.float32)
    bv = np.asarray(bv, dtype=np.float32)
    Wo = np.asarray(Wo, dtype=np.float32)
    bo = np.asarray(bo, dtype=np.float32)
    attn_mask = np.asarray(attn_mask)
    key_padding_mask = np.asarray(key_padding_mask)

    # this kernel hardcodes the causal structure of attn_mask
    expected = np.triu(np.ones((S, S), dtype=bool), k=1)
    assert np.array_equal(attn_mask, expected), "kernel assumes causal attn_mask"

    # number of 128-blocks that contain any valid (unpadded) key
    valid = ~key_padding_mask  # [B, S]
    kb_cap = 0
    for b in range(B):
        nz = np.nonzero(valid[b])[0]
        cap = (int(nz.max()) // 128 + 1) if nz.size else 1
        kb_cap = max(kb_cap, cap)

    scale = np.float32(1.0 / np.sqrt(DK))
    causal_tile = np.where(
        np.arange(128)[None, :] >= np.arange(128)[:, None], 0.0, -60000.0
    ).astype(np.float16)
    causal2 = np.ascontiguousarray(
        np.stack([causal_tile, causal_tile], axis=1)
    )  # [128, 2, 128]

    # per-batch transposed activations (shared by the batch's 4 cores)
    xq_b = [np.ascontiguousarray(query[:, b, :].T.astype(np.float16)) for b in range(B)]
    xk_b = [np.ascontiguousarray(key[:, b, :].T.astype(np.float16)) for b in range(B)]
    xv_b = [np.ascontiguousarray(value[:, b, :].T.astype(np.float16)) for b in range(B)]
    pad_b = [
        np.ascontiguousarray(
            np.where(key_padding_mask[b], NEG, 0.0)
            .astype(np.float32)
            .reshape(KB, 128)
            .T
        )
        for b in range(B)
    ]

    in_maps = []
    for c in range(N_CORES):
        b = c // GROUPS
        g = c % GROUPS
        o0 = g * OC
        osl = slice(o0, o0 + OC)
        bias_qk = np.stack(
            [
                (bq[osl] * scale).reshape(OT, 128).T,
                bk[osl].reshape(OT, 128).T,
            ],
            axis=1,
        ).astype(np.float32)  # [128, 2, OT]
        in_maps.append(
            {
                "xq": xq_b[b],
                "xk": xk_b[b],
                "xv": xv_b[b],
                "wq": np.ascontiguousarray((Wq[osl, :] * scale).T.astype(np.float16)),
                "wk": np.ascontiguousarray(Wk[osl, :].T.astype(np.float16)),
                "wv": np.ascontiguousarray(Wv[osl, :].T.astype(np.float16)),
                "wo": np.ascontiguousarray(Wo[:, osl].T).astype(np.float16),
                "bias_qk": np.ascontiguousarray(bias_qk),
                "bias_v": np.ascontiguousarray(bv[osl][None, :]),
                "pad": pad_b[b],
                "causal2": causal2,
            }
        )

    res = bass_utils.run_bass_kernel_spmd(
        _get_nc(kb_cap), in_maps, core_ids=list(range(N_CORES))
    )
    _cache["last_res"] = res

    out = np.zeros((S, B, D), dtype=np.float32)
    for b in range(B):
        acc = np.zeros((D, S), dtype=np.float32)
        for g in range(GROUPS):
            acc += res.results[b * GROUPS + g]["out_t"].astype(np.float32)
        out[:, b, :] = acc.T + bo[None, :]
    return out


# revision 10
# speedup vs baseline: 1.0733x; 1.0312x over previous
"""Trainium2 Bass kernel for nn_MultiHeadAttention_59614146068609.

Sharding: 8 cores = 2 batches x 4 head-groups (4 heads each).
Each core projects q/k/v for its batch with its head-slice of Wq/Wk/Wv
(column-sharded), runs causal+padded attention for its 4 heads, and
applies its row-slice of Wo, producing a partial [D, S] fp16 output.
The host sums the 4 partials per batch and adds bo (with Wo @ bv folded
in on the host: softmax weights sum to 1, so attn(v + bv) = attn(v) + bv).

Layout: q/k land transposed and PAIR-PACKED ([128, pair, s] with head
2p in partitions 0:64 and head 2p+1 in 64:128), so projections evict
full-width tiles and attention runs 64-contraction matmuls at partition
offsets 0/64 (PE quadrant tile_position). Scores for a head-pair land
in one 2-bank PSUM tile [128, 2, 512] so a single ACT exp instruction
covers both heads (the scalar engine is the scarce resource). V is
natural layout with an appended ones-column providing softmax sums.
The causal mask is applied AFTER exp as a 0/1 multiply on the fp16
probability tile, keeping the vector engine out of the scores->exp
critical chain.

Schedule: one software pipeline. Attention for chunk qc interleaves,
per key-block step, "filler" tensor work units (q/k/v projections for
qc+1 and the Wo output projection of qc-1) popped from a queue, so the
tensor engine always streams (TRN2 PE p-state needs continuous
execution for 2.4 GHz). PV trails scores by one key block. All inputs
are host-prepacked so every DMA is per-partition contiguous.

Specialized at build time on kb_cap = number of 128-wide key blocks
that contain any unpadded key; fully padded key blocks are skipped.
"""

from collections import deque

import numpy as np

S = 2048
B = 2
D = 1024
H = 16
DK = 64
N_CORES = 8
GROUPS = N_CORES // B          # head groups per batch = 4
HPG = H // GROUPS              # heads per group = 4
OC = HPG * DK                  # per-core projected dim = 256
OT = OC // 128                 # o-tiles / head-pairs per core = 2
IT = D // 128                  # contraction tiles = 8
SC = S // 512                  # sequence chunks of 512 = 4
KB = S // 128                  # k blocks of 128 = 16
NEG = -1e30

_cache = {}


def _build_nc(kb_cap):
    import concourse.bacc as bacc
    import concourse.bass as bass
    import concourse.mybir as mybir
    import concourse.tile as tile
    from concourse import library_config

    F32 = mybir.dt.float32
    FP16 = mybir.dt.float16
    Exp = mybir.ActivationFunctionType.Exp
    PSUM = bass.MemorySpace.PSUM

    ksc = -(-kb_cap * 128 // 512)        # 512-chunks of k to project
    nkb = [min(4 * (qc + 1), kb_cap) for qc in range(SC)]

    def vblocks(qc):
        # v key-blocks first needed by attention chunk qc
        if qc >= SC:
            return []
        return list(range(4 * qc, min(4 * (qc + 1), kb_cap)))

    nc = bacc.Bacc("TRN2", target_bir_lowering=False, debug=False)

    # all inputs host-prepacked: partition-major, chunk-contiguous
    xq = nc.dram_tensor("xq", [128, SC, IT, 512], FP16, kind="ExternalInput")
    xk = nc.dram_tensor("xk", [128, SC, IT, 512], FP16, kind="ExternalInput")
    xv = nc.dram_tensor("xv", [128, SC, IT, 512], FP16, kind="ExternalInput")
    wq = nc.dram_tensor("wq", [128, IT, OC], FP16, kind="ExternalInput")
    wk = nc.dram_tensor("wk", [128, IT, OC], FP16, kind="ExternalInput")
    wv = nc.dram_tensor("wv", [128, IT, OC], FP16, kind="ExternalInput")
    wo = nc.dram_tensor("wo", [128, OT, D], FP16, kind="ExternalInput")
    # consts: cols 0:2 = scaled bq (per o-tile), 2:4 = bk, 4:20 = pad bias
    consts = nc.dram_tensor("consts", [128, 20], F32, kind="ExternalInput")
    mask01 = nc.dram_tensor("mask01", [128, 2, 128], FP16, kind="ExternalInput")
    out_t = nc.dram_tensor("out_t", [D, S], FP16, kind="ExternalOutput")

    with tile.TileContext(nc) as tc, nc.allow_low_precision(
        reason="fp16 compute throughout; validated vs fp32 reference"
    ):
        with (
            tc.tile_pool(name="persist", bufs=1) as pp,
            tc.tile_pool(name="xs", bufs=2) as xs,
            tc.tile_pool(name="ptp", bufs=6) as ptp,
            tc.tile_pool(name="nrm", bufs=2) as nrm,
            tc.tile_pool(name="stg", bufs=4) as stg,
            tc.tile_pool(name="ps", bufs=2, space=PSUM) as ps,
        ):
            nc.gpsimd.load_library(library_config.attn)

            # ---- persistent SBUF tensors ----
            t_wq = pp.tile([128, IT, OC], FP16)
            t_wk = pp.tile([128, IT, OC], FP16)
            t_wv = pp.tile([128, IT, OC], FP16)
            t_wo = pp.tile([128, OT, D], FP16)
            t_c = pp.tile([128, 20], F32)
            t_mask = pp.tile([128, 2, 128], FP16)
            t_qT = pp.tile([128, OT, S], FP16)
            t_kT = pp.tile([128, OT, ksc * 512], FP16)
            t_V = pp.tile([128, kb_cap, HPG, DK + 1], FP16)
            t_OT = pp.tile([128, OT, S], FP16)

            # weights on the scalar queue (ACT idle early); consts on vector
            nc.scalar.dma_start(out=t_wq, in_=wq[:])
            nc.scalar.dma_start(out=t_wk, in_=wk[:])
            nc.scalar.dma_start(out=t_wv, in_=wv[:])
            nc.scalar.dma_start(out=t_wo, in_=wo[:])
            nc.sync.dma_start(out=t_c, in_=consts[:])
            nc.sync.dma_start(out=t_mask, in_=mask01[:])
            # softmax-denominator ones column of V
            nc.vector.memset(t_V[:, :, :, DK : DK + 1], 1.0)

            # ---- chunk-granular x DMAs (sync queue) ----
            xq_t = {}
            xk_t = {}
            xv_t = {}

            def dma_xchunk(sc):
                if sc < SC:
                    t = xs.tile([128, IT, 512], FP16, tag="xq", name=f"xq_{sc}")
                    nc.sync.dma_start(out=t, in_=xq[:, sc, :, :])
                    xq_t[sc] = t
                if sc < ksc:
                    t = xs.tile([128, IT, 512], FP16, tag="xk", name=f"xk_{sc}")
                    nc.sync.dma_start(out=t, in_=xk[:, sc, :, :])
                    xk_t[sc] = t

            def dma_xv(blocks):
                if not blocks:
                    return
                g = blocks[0] // 4
                t = xs.tile([128, IT, 512], FP16, tag="xv", name=f"xv_{g}")
                nc.sync.dma_start(out=t, in_=xv[:, g, :, :])
                xv_t[g] = t

            # ---- work units (each emits one PE matmul group + eviction) ----
            def unit_qk(sc, w_sb, cofs, dst, ot):
                def emit():
                    xt = xq_t[sc] if cofs == 0 else xk_t[sc]
                    acc = ps.tile(
                        [128, 2, 512], F32, tag="w", bufs=2, name=f"a{cofs}_{sc}_{ot}"
                    )
                    for i in range(IT):
                        nc.tensor.matmul(
                            acc[:, 0, :],
                            w_sb[:, i, ot * 128 : (ot + 1) * 128],
                            xt[:, i, :],
                            start=(i == 0),
                            stop=(i == IT - 1),
                        )
                    nc.vector.tensor_scalar_add(
                        out=dst[:, ot, sc * 512 : (sc + 1) * 512],
                        in0=acc[:, 0, :],
                        scalar1=t_c[:, cofs + ot : cofs + ot + 1],
                    )
                return emit

            def unit_v(n, blk):
                def emit():
                    xt = xv_t[blk // 4]
                    vacc = ps.tile([128, 2, 512], F32, tag="w", bufs=2, name=f"v_{blk}")
                    for i in range(IT):
                        nc.tensor.matmul(
                            vacc[:, 0, 0:OC],
                            xt[:, i, n * 128 : (n + 1) * 128],
                            t_wv[:, i, :],
                            start=(i == 0),
                            stop=(i == IT - 1),
                        )
                    nc.vector.tensor_copy(
                        out=t_V[:, blk, :, 0:DK],
                        in_=vacc[:, 0, 0:OC].rearrange("p (h d) -> p h d", h=HPG),
                    )
                return emit

            def unit_phase_c(qc, dt_):
                def emit():
                    q0 = qc * 512
                    ops = ps.tile(
                        [128, 2, 512], F32, tag="w", bufs=2, name=f"c_{qc}_{dt_}"
                    )
                    for j in range(OT):
                        nc.tensor.matmul(
                            ops[:, 0, :],
                            t_wo[:, j, dt_ * 128 : (dt_ + 1) * 128],
                            t_OT[:, j, q0 : q0 + 512],
                            start=(j == 0),
                            stop=(j == OT - 1),
                        )
                    st_o = stg.tile([128, 512], FP16, tag="s", name=f"so_{qc}_{dt_}")
                    nc.vector.tensor_copy(st_o, ops[:, 0, :])
                    nc.gpsimd.dma_start(
                        out=out_t[dt_ * 128 : (dt_ + 1) * 128, q0 : q0 + 512],
                        in_=st_o,
                    )
                return emit

            def proj_units(sc):
                u = []
                if sc < SC:
                    for ot in range(OT):
                        u.append(unit_qk(sc, t_wq, 0, t_qT, ot))
                if sc < ksc:
                    for ot in range(OT):
                        u.append(unit_qk(sc, t_wk, 2, t_kT, ot))
                for n, blk in enumerate(vblocks(sc)):
                    u.append(unit_v(n, blk))
                return u

            # ---- fused pipelined main loop ----
            dma_xchunk(0)
            dma_xv(vblocks(0))
            dma_xchunk(1)
            dma_xv(vblocks(1))

            fillers = deque()
            # chunk 0's q/k projections must precede its first scores matmul;
            # its v projections interleave into the loop (PV trails 1 block)
            fillers.extend(proj_units(0))
            n_fill0 = 2 * OT if ksc > 0 else OT

            for qc in range(SC):
                q0 = qc * 512
                # prefetch chunk qc+2 inputs; queue chunk qc+1 projections
                dma_xchunk(qc + 2)
                dma_xv(vblocks(qc + 2))
                if qc + 1 < SC:
                    fillers.extend(proj_units(qc + 1))

                o_ps = {
                    (p, hh): ps.tile(
                        [128, 512], F32, tag="o", bufs=4, name=f"o_{qc}_{p}_{hh}"
                    )
                    for p in range(OT)
                    for hh in range(2)
                }
                last = nkb[qc] - 1

                def emit_pv(kb, off, pts, o_ps=o_ps, last=last):
                    for p in range(OT):
                        for hh in range(2):
                            nc.tensor.matmul(
                                o_ps[(p, hh)][0 : DK + 1, off:512],
                                t_V[:, kb, 2 * p + hh, :],
                                pts[p][:, hh, off:512],
                                start=(kb == 0),
                                stop=(kb == last),
                            )

                if qc == 0:
                    for _ in range(n_fill0):
                        fillers.popleft()()

                steps = nkb[qc]
                prev = None
                for kb in range(steps):
                    k0 = kb * 128
                    off = max(0, k0 - q0)
                    st2s = {}
                    for p in range(OT):
                        st2 = ps.tile(
                            [128, 2, 512], F32, tag="w", bufs=2,
                            name=f"st_{qc}_{kb}_{p}",
                        )
                        for hh in range(2):
                            nc.tensor.matmul(
                                st2[:, hh, off:512],
                                t_kT[hh * 64 : (hh + 1) * 64, p, k0 : k0 + 128],
                                t_qT[hh * 64 : (hh + 1) * 64, p, q0 + off : q0 + 512],
                                start=True,
                                stop=True,
                            )
                        st2s[p] = st2
                    if prev is not None:
                        emit_pv(*prev)
                    pts = {}
                    for p in range(OT):
                        pt = ptp.tile(
                            [128, 2, 512], FP16, tag="pt", name=f"pt_{qc}_{kb}_{p}"
                        )
                        nc.scalar.activation(
                            out=pt[:, :, off:512],
                            in_=st2s[p][:, :, off:512],
                            func=Exp,
                            bias=t_c[:, 4 + kb : 5 + kb],
                            scale=1.0,
                        )
                        pts[p] = pt
                    if k0 >= q0:
                        # causal mask applied post-exp (0/1 multiply) so the
                        # vector engine stays out of the scores->exp chain
                        for p in range(OT):
                            nc.vector.tensor_mul(
                                pts[p][:, :, off : off + 128],
                                pts[p][:, :, off : off + 128],
                                t_mask,
                            )
                    # interleave filler tensor work (emitted AFTER this
                    # block's exp so PSUM slot rotation can never order a
                    # filler's eviction ahead of the exp chain that releases
                    # its slot)
                    remaining = steps - kb
                    n_pop = max(1, -(-len(fillers) // remaining)) if fillers else 0
                    for _ in range(min(n_pop, len(fillers))):
                        fillers.popleft()()
                    prev = (kb, off, pts)
                emit_pv(*prev)

                # normalize by the ones-column sums -> t_OT
                for p in range(OT):
                    for hh in range(2):
                        t_l = nrm.tile([1, 512], F32, tag="l", name=f"l_{qc}_{p}_{hh}")
                        nc.vector.tensor_copy(t_l[0:1, :], o_ps[(p, hh)][DK : DK + 1, :])
                        t_r = nrm.tile([1, 512], F32, tag="r", name=f"r_{qc}_{p}_{hh}")
                        nc.vector.reciprocal_approx_fast(t_r[0:1, :], t_l[0:1, :])
                        t_rb = nrm.tile(
                            [DK, 512], F32, tag="rb", name=f"rb_{qc}_{p}_{hh}"
                        )
                        nc.gpsimd.partition_broadcast(t_rb, t_r[0:1, :])
                        nc.vector.tensor_mul(
                            t_OT[hh * 64 : (hh + 1) * 64, p, q0 : q0 + 512],
                            o_ps[(p, hh)][0:DK, :],
                            t_rb,
                        )

                # output projection of this chunk runs interleaved into the
                # next chunk's attention; the final chunk's runs at the tail
                if qc + 1 < SC:
                    fillers.extend(unit_phase_c(qc, dt_) for dt_ in range(D // 128))
                else:
                    for dt_ in range(D // 128):
                        unit_phase_c(qc, dt_)()
    nc.compile()
    return nc


def _get_nc(kb_cap):
    key = ("nc", kb_cap)
    if key not in _cache:
        _cache[key] = _build_nc(kb_cap)
    return _cache[key]


def _prepack_x(x):  # [D, S] -> [128, SC, IT, 512], partition-major chunks
    return np.ascontiguousarray(
        x.reshape(IT, 128, SC, 512).transpose(1, 2, 0, 3)
    )


def kernel(
    query,
    key,
    value,
    Wq,
    bq,
    Wk,
    bk,
    Wv,
    bv,
    Wo,
    bo,
    attn_mask,
    key_padding_mask,
):
    from concourse import bass_utils

    query = np.asarray(query, dtype=np.float32)
    key = np.asarray(key, dtype=np.float32)
    value = np.asarray(value, dtype=np.float32)
    Wq = np.asarray(Wq, dtype=np.float32)
    bq = np.asarray(bq, dtype=np.float32)
    Wk = np.asarray(Wk, dtype=np.float32)
    bk = np.asarray(bk, dtype=np.float32)
    Wv = np.asarray(Wv, dtype=np.float32)
    bv = np.asarray(bv, dtype=np.float32)
    Wo = np.asarray(Wo, dtype=np.float32)
    bo = np.asarray(bo, dtype=np.float32)
    attn_mask = np.asarray(attn_mask)
    key_padding_mask = np.asarray(key_padding_mask)

    # this kernel hardcodes the causal structure of attn_mask
    expected = np.triu(np.ones((S, S), dtype=bool), k=1)
    assert np.array_equal(attn_mask, expected), "kernel assumes causal attn_mask"

    # number of 128-blocks that contain any valid (unpadded) key
    valid = ~key_padding_mask  # [B, S]
    kb_cap = 0
    for b in range(B):
        nz = np.nonzero(valid[b])[0]
        cap = (int(nz.max()) // 128 + 1) if nz.size else 1
        kb_cap = max(kb_cap, cap)

    scale = np.float32(1.0 / np.sqrt(DK))
    m01 = (np.arange(128)[None, :] >= np.arange(128)[:, None]).astype(np.float16)
    mask01 = np.ascontiguousarray(np.stack([m01, m01], axis=1))  # [128, 2, 128]

    # per-batch prepacked activations (shared by the batch's 4 cores)
    xq_b = [_prepack_x(query[:, b, :].T.astype(np.float16)) for b in range(B)]
    xk_b = [_prepack_x(key[:, b, :].T.astype(np.float16)) for b in range(B)]
    xv_b = [_prepack_x(value[:, b, :].T.astype(np.float16)) for b in range(B)]
    pad_b = [
        np.where(key_padding_mask[b], NEG, 0.0).astype(np.float32).reshape(KB, 128).T
        for b in range(B)
    ]

    in_maps = []
    for c in range(N_CORES):
        b = c // GROUPS
        g = c % GROUPS
        o0 = g * OC
        osl = slice(o0, o0 + OC)
        consts = np.zeros((128, 20), dtype=np.float32)
        consts[:, 0:2] = (bq[osl] * scale).reshape(OT, 128).T
        consts[:, 2:4] = bk[osl].reshape(OT, 128).T
        consts[:, 4:20] = pad_b[b]
        in_maps.append(
            {
                "xq": xq_b[b],
                "xk": xk_b[b],
                "xv": xv_b[b],
                "wq": np.ascontiguousarray(
                    (Wq[osl, :] * scale).T.astype(np.float16)
                    .reshape(IT, 128, OC).transpose(1, 0, 2)
                ),
                "wk": np.ascontiguousarray(
                    Wk[osl, :].T.astype(np.float16)
                    .reshape(IT, 128, OC).transpose(1, 0, 2)
                ),
                "wv": np.ascontiguousarray(
                    Wv[osl, :].T.astype(np.float16)
                    .reshape(IT, 128, OC).transpose(1, 0, 2)
                ),
                "wo": np.ascontiguousarray(
                    Wo[:, osl].T.astype(np.float16)
                    .reshape(OT, 128, D).transpose(1, 0, 2)
                ),
                "consts": consts,
                "mask01": mask01,
            }
        )

    res = bass_utils.run_bass_kernel_spmd(
        _get_nc(kb_cap), in_maps, core_ids=list(range(N_CORES))
    )
    _cache["last_res"] = res

    bo2 = bo + Wo @ bv  # softmax weights sum to 1: attn(v+bv) = attn(v)+bv
    out = np.zeros((S, B, D), dtype=np.float32)
    for b in range(B):
        acc = np.zeros((D, S), dtype=np.float32)
        for g in range(GROUPS):
            acc += res.results[b * GROUPS + g]["out_t"].astype(np.float32)
        out[:, b, :] = acc.T + bo2[None, :]
    return out


# revision 16
# speedup vs baseline: 1.1087x; 1.0330x over previous
"""Trainium2 Bass kernel for nn_MultiHeadAttention_59614146068609.

Sharding: 8 cores = 2 batches x 4 head-groups (4 heads each).
Each core projects q/k/v for its batch with its head-slice of Wq/Wk/Wv
(column-sharded), runs causal+padded attention for its 4 heads, and
applies its row-slice of Wo, producing a partial [D, S] fp16 output.
The host sums the 4 partials per batch and adds bo (with Wo @ bv folded
in on the host: softmax weights sum to 1, so attn(v + bv) = attn(v) + bv).

Layout: q/k land transposed and PAIR-PACKED ([128, pair, s] with head
2p in partitions 0:64 and head 2p+1 in 64:128), so projections evict
full-width tiles and attention runs 64-contraction matmuls at partition
offsets 0/64 (PE quadrant tile_position). Scores for a head-pair land
in one 2-bank PSUM tile [128, 2, 512] so a single ACT exp instruction
covers both heads (the scalar engine is the scarce resource). V is
natural layout with an appended ones-column providing softmax sums.
The causal mask is applied AFTER exp as a 0/1 multiply on the fp16
probability tile, keeping the vector engine out of the scores->exp
critical chain.

Schedule: one software pipeline. Attention for chunk qc interleaves,
per key-block step, "filler" tensor work units (q/k/v projections for
qc+1 and the Wo output projection of qc-1) popped from a queue, so the
tensor engine always streams (TRN2 PE p-state needs continuous
execution for 2.4 GHz). PV trails scores by one key block. All inputs
are host-prepacked so every DMA is per-partition contiguous.

Specialized at build time on kb_cap = number of 128-wide key blocks
that contain any unpadded key; fully padded key blocks are skipped.
"""

from collections import deque

import numpy as np

S = 2048
B = 2
D = 1024
H = 16
DK = 64
N_CORES = 8
GROUPS = N_CORES // B          # head groups per batch = 4
HPG = H // GROUPS              # heads per group = 4
OC = HPG * DK                  # per-core projected dim = 256
OT = OC // 128                 # o-tiles / head-pairs per core = 2
IT = D // 128                  # contraction tiles = 8
SC = S // 512                  # sequence chunks of 512 = 4
KB = S // 128                  # k blocks of 128 = 16
NEG = -1e30

_cache = {}


def _build_nc(kb_cap):
    import concourse.bacc as bacc
    import concourse.bass as bass
    import concourse.mybir as mybir
    import concourse.tile as tile
    from concourse import library_config

    F32 = mybir.dt.float32
    FP16 = mybir.dt.float16
    Exp = mybir.ActivationFunctionType.Exp
    PSUM = bass.MemorySpace.PSUM

    ksc = -(-kb_cap * 128 // 512)        # 512-chunks of k to project
    nkb = [min(4 * (qc + 1), kb_cap) for qc in range(SC)]

    def vblocks(qc):
        # v key-blocks first needed by attention chunk qc
        if qc >= SC:
            return []
        return list(range(4 * qc, min(4 * (qc + 1), kb_cap)))

    nc = bacc.Bacc("TRN2", target_bir_lowering=False, debug=False)

    # all inputs host-prepacked: partition-major, chunk-contiguous
    xq = nc.dram_tensor("xq", [128, SC, IT, 512], FP16, kind="ExternalInput")
    xk = nc.dram_tensor("xk", [128, SC, IT, 512], FP16, kind="ExternalInput")
    xv = nc.dram_tensor("xv", [128, SC, IT, 512], FP16, kind="ExternalInput")
    wq = nc.dram_tensor("wq", [128, IT, OC], FP16, kind="ExternalInput")
    wk = nc.dram_tensor("wk", [128, IT, OC], FP16, kind="ExternalInput")
    wv = nc.dram_tensor("wv", [128, IT, OC], FP16, kind="ExternalInput")
    wo = nc.dram_tensor("wo", [128, OT, D], FP16, kind="ExternalInput")
    # consts: cols 0:2 = scaled bq (per o-tile), 2:4 = bk, 4:20 = pad bias
    consts = nc.dram_tensor("consts", [128, 20], F32, kind="ExternalInput")
    mask01 = nc.dram_tensor("mask01", [128, 2, 128], FP16, kind="ExternalInput")
    out_t = nc.dram_tensor("out_t", [D, S], FP16, kind="ExternalOutput")

    with tile.TileContext(nc) as tc, nc.allow_low_precision(
        reason="fp16 compute throughout; validated vs fp32 reference"
    ):
        with (
            tc.tile_pool(name="persist", bufs=1) as pp,
            tc.tile_pool(name="xs", bufs=2) as xs,
            tc.tile_pool(name="ptp", bufs=6) as ptp,
            tc.tile_pool(name="nrm", bufs=2) as nrm,
            tc.tile_pool(name="stg", bufs=4) as stg,
            tc.tile_pool(name="ps", bufs=2, space=PSUM) as ps,
        ):
            nc.gpsimd.load_library(library_config.attn)

            # ---- persistent SBUF tensors ----
            t_wq = pp.tile([128, IT, OC], FP16)
            t_wk = pp.tile([128, IT, OC], FP16)
            t_wv = pp.tile([128, IT, OC], FP16)
            t_wo = pp.tile([128, OT, D], FP16)
            t_c = pp.tile([128, 20], F32)
            t_mask = pp.tile([128, 2, 128], FP16)
            t_qT = pp.tile([128, OT, S], FP16)
            t_kT = pp.tile([128, OT, ksc * 512], FP16)
            t_V = pp.tile([128, kb_cap, HPG, DK + 1], FP16)
            t_OT = pp.tile([128, OT, S], FP16)

            # DMA load split across all three issuing queues (each sprays
            # over 16 DMA engines): sync=xq+consts, scalar=weights+xk,
            # gpsimd=xv+output. Ordered by first use.
            nc.scalar.dma_start(out=t_wq, in_=wq[:])
            nc.scalar.dma_start(out=t_wk, in_=wk[:])
            nc.sync.dma_start(out=t_c, in_=consts[:])
            nc.sync.dma_start(out=t_mask, in_=mask01[:])
            # softmax-denominator ones column of V
            nc.vector.memset(t_V[:, :, :, DK : DK + 1], 1.0)

            # ---- chunk-granular x DMAs (sync queue) ----
            xq_t = {}
            xk_t = {}
            xv_t = {}

            def dma_xchunk(sc):
                if sc < SC:
                    t = xs.tile([128, IT, 512], FP16, tag="xq", name=f"xq_{sc}")
                    nc.sync.dma_start(out=t, in_=xq[:, sc, :, :])
                    xq_t[sc] = t
                if sc < ksc:
                    t = xs.tile([128, IT, 512], FP16, tag="xk", name=f"xk_{sc}")
                    nc.scalar.dma_start(out=t, in_=xk[:, sc, :, :])
                    xk_t[sc] = t
                if sc == 0:
                    # remaining weights queue behind wq/wk/xk0 on scalar
                    nc.scalar.dma_start(out=t_wv, in_=wv[:])
                    nc.scalar.dma_start(out=t_wo, in_=wo[:])

            def dma_xv(blocks):
                if not blocks:
                    return
                g = blocks[0] // 4
                t = xs.tile([128, IT, 512], FP16, tag="xv", name=f"xv_{g}")
                nc.gpsimd.dma_start(out=t, in_=xv[:, g, :, :])
                xv_t[g] = t

            # ---- work units (each emits one PE matmul group + eviction) ----
            def unit_qk(sc, w_sb, cofs, dst, ot):
                def emit():
                    xt = xq_t[sc] if cofs == 0 else xk_t[sc]
                    acc = ps.tile(
                        [128, 2, 512], F32, tag="w", bufs=2, name=f"a{cofs}_{sc}_{ot}"
                    )
                    for i in range(IT):
                        nc.tensor.matmul(
                            acc[:, 0, :],
                            w_sb[:, i, ot * 128 : (ot + 1) * 128],
                            xt[:, i, :],
                            start=(i == 0),
                            stop=(i == IT - 1),
                        )
                    nc.vector.tensor_scalar_add(
                        out=dst[:, ot, sc * 512 : (sc + 1) * 512],
                        in0=acc[:, 0, :],
                        scalar1=t_c[:, cofs + ot : cofs + ot + 1],
                    )
                return emit

            def unit_v(n, blk):
                def emit():
                    xt = xv_t[blk // 4]
                    vacc = ps.tile([128, 2, 512], F32, tag="w", bufs=2, name=f"v_{blk}")
                    for i in range(IT):
                        nc.tensor.matmul(
                            vacc[:, 0, 0:OC],
                            xt[:, i, n * 128 : (n + 1) * 128],
                            t_wv[:, i, :],
                            start=(i == 0),
                            stop=(i == IT - 1),
                        )
                    nc.vector.tensor_copy(
                        out=t_V[:, blk, :, 0:DK],
                        in_=vacc[:, 0, 0:OC].rearrange("p (h d) -> p h d", h=HPG),
                    )
                return emit

            def unit_phase_c(qc, dt_):
                def emit():
                    q0 = qc * 512
                    ops = ps.tile(
                        [128, 2, 512], F32, tag="w", bufs=2, name=f"c_{qc}_{dt_}"
                    )
                    for j in range(OT):
                        nc.tensor.matmul(
                            ops[:, 0, :],
                            t_wo[:, j, dt_ * 128 : (dt_ + 1) * 128],
                            t_OT[:, j, q0 : q0 + 512],
                            start=(j == 0),
                            stop=(j == OT - 1),
                        )
                    st_o = stg.tile([128, 512], FP16, tag="s", name=f"so_{qc}_{dt_}")
                    nc.vector.tensor_copy(st_o, ops[:, 0, :])
                    nc.gpsimd.dma_start(
                        out=out_t[dt_ * 128 : (dt_ + 1) * 128, q0 : q0 + 512],
                        in_=st_o,
                    )
                return emit

            def proj_units(sc):
                u = []
                if sc < SC:
                    for ot in range(OT):
                        u.append(unit_qk(sc, t_wq, 0, t_qT, ot))
                if sc < ksc:
                    for ot in range(OT):
                        u.append(unit_qk(sc, t_wk, 2, t_kT, ot))
                for n, blk in enumerate(vblocks(sc)):
                    u.append(unit_v(n, blk))
                return u

            # ---- fused pipelined main loop ----
            dma_xchunk(0)
            dma_xv(vblocks(0))
            dma_xchunk(1)
            dma_xv(vblocks(1))

            fillers = deque()
            # chunk 0's q/k projections must precede its first scores matmul;
            # its v projections interleave into the loop (PV trails 1 block)
            fillers.extend(proj_units(0))
            n_fill0 = 2 * OT if ksc > 0 else OT

            for qc in range(SC):
                q0 = qc * 512
                # prefetch chunk qc+2 inputs; queue chunk qc+1 projections
                dma_xchunk(qc + 2)
                dma_xv(vblocks(qc + 2))
                if qc + 1 < SC:
                    if qc == 0:
                        # chunk-0's v units are still queued and must pop
                        # first (PV of chunk 0 trails them by one step)
                        fillers.extend(proj_units(1))
                    else:
                        # projections go to the FRONT: their inputs (x
                        # chunks) are long since resident, while the phase-C
                        # units queued behind depend on this chunk's norm
                        # and would head-of-line block the PE if popped first
                        fillers.extendleft(reversed(proj_units(qc + 1)))

                o_ps = {
                    (p, hh): ps.tile(
                        [128, 512], F32, tag="o", bufs=4, name=f"o_{qc}_{p}_{hh}"
                    )
                    for p in range(OT)
                    for hh in range(2)
                }
                last = nkb[qc] - 1

                def emit_pv(kb, off, pts, o_ps=o_ps, last=last):
                    for p in range(OT):
                        for hh in range(2):
                            nc.tensor.matmul(
                                o_ps[(p, hh)][0 : DK + 1, off:512],
                                t_V[:, kb, 2 * p + hh, :],
                                pts[p][:, hh, off:512],
                                start=(kb == 0),
                                stop=(kb == last),
                            )

                if qc == 0:
                    for _ in range(n_fill0):
                        fillers.popleft()()

                steps = nkb[qc]
                prev = None
                for kb in range(steps):
                    k0 = kb * 128
                    off = max(0, k0 - q0)
                    st2s = {}
                    for p in range(OT):
                        st2 = ps.tile(
                            [128, 2, 512], F32, tag="w", bufs=2,
                            name=f"st_{qc}_{kb}_{p}",
                        )
                        for hh in range(2):
                            nc.tensor.matmul(
                                st2[:, hh, off:512],
                                t_kT[hh * 64 : (hh + 1) * 64, p, k0 : k0 + 128],
                                t_qT[hh * 64 : (hh + 1) * 64, p, q0 + off : q0 + 512],
                                start=True,
                                stop=True,
                            )
                        st2s[p] = st2
                    if prev is not None:
                        emit_pv(*prev)
                    pts = {}
                    for p in range(OT):
                        pt = ptp.tile(
                            [128, 2, 512], FP16, tag="pt", name=f"pt_{qc}_{kb}_{p}"
                        )
                        nc.scalar.activation(
                            out=pt[:, :, off:512],
                            in_=st2s[p][:, :, off:512],
                            func=Exp,
                            bias=t_c[:, 4 + kb : 5 + kb],
                            scale=1.0,
                        )
                        pts[p] = pt
                    if k0 >= q0:
                        # causal mask applied post-exp (0/1 multiply) so the
                        # vector engine stays out of the scores->exp chain
                        for p in range(OT):
                            nc.vector.tensor_mul(
                                pts[p][:, :, off : off + 128],
                                pts[p][:, :, off : off + 128],
                                t_mask,
                            )
                    # interleave filler tensor work (emitted AFTER this
                    # block's exp so PSUM slot rotation can never order a
                    # filler's eviction ahead of the exp chain that releases
                    # its slot)
                    remaining = steps - kb
                    n_pop = max(1, -(-len(fillers) // remaining)) if fillers else 0
                    for _ in range(min(n_pop, len(fillers))):
                        fillers.popleft()()
                    prev = (kb, off, pts)
                emit_pv(*prev)

                # normalize by the ones-column sums -> t_OT
                for p in range(OT):
                    for hh in range(2):
                        t_l = nrm.tile([1, 512], F32, tag="l", name=f"l_{qc}_{p}_{hh}")
                        nc.vector.tensor_copy(t_l[0:1, :], o_ps[(p, hh)][DK : DK + 1, :])
                        t_r = nrm.tile([1, 512], F32, tag="r", name=f"r_{qc}_{p}_{hh}")
                        nc.vector.reciprocal_approx_fast(t_r[0:1, :], t_l[0:1, :])
                        t_rb = nrm.tile(
                            [DK, 512], F32, tag="rb", name=f"rb_{qc}_{p}_{hh}"
                        )
                        nc.gpsimd.partition_broadcast(t_rb, t_r[0:1, :])
                        nc.vector.tensor_mul(
                            t_OT[hh * 64 : (hh + 1) * 64, p, q0 : q0 + 512],
                            o_ps[(p, hh)][0:DK, :],
                            t_rb,
                        )

                # output projection of this chunk runs interleaved into the
                # next chunk's attention; the final chunk's runs at the tail
                if qc + 1 < SC:
                    fillers.extend(unit_phase_c(qc, dt_) for dt_ in range(D // 128))
                else:
                    for dt_ in range(D // 128):
                        unit_phase_c(qc, dt_)()
    nc.compile()
    return nc


def _get_nc(kb_cap):
    key = ("nc", kb_cap)
    if key not in _cache:
        _cache[key] = _build_nc(kb_cap)
    return _cache[key]


def _prepack_x(x):  # [D, S] -> [128, SC, IT, 512], partition-major chunks
    return np.ascontiguousarray(
        x.reshape(IT, 128, SC, 512).transpose(1, 2, 0, 3)
    )


def kernel(
    query,
    key,
    value,
    Wq,
    bq,
    Wk,
    bk,
    Wv,
    bv,
    Wo,
    bo,
    attn_mask,
    key_padding_mask,
):
    from concourse import bass_utils

    query = np.asarray(query, dtype=np.float32)
    key = np.asarray(key, dtype=np.float32)
    value = np.asarray(value, dtype=np.float32)
    Wq = np.asarray(Wq, dtype=np.float32)
    bq = np.asarray(bq, dtype=np.float32)
    Wk = np.asarray(Wk, dtype=np.float32)
    bk = np.asarray(bk, dtype=np.float32)
    Wv = np.asarray(Wv, dtype=np.float32)
    bv = np.asarray(bv, dtype=np.float32)
    Wo = np.asarray(Wo, dtype=np.float32)
    bo = np.asarray(bo, dtype=np.float32)
    attn_mask = np.asarray(attn_mask)
    key_padding_mask = np.asarray(key_padding_mask)

    # this kernel hardcodes the causal structure of attn_mask
    expected = np.triu(np.ones((S, S), dtype=bool), k=1)
    assert np.array_equal(attn_mask, expected), "kernel assumes causal attn_mask"

    # number of 128-blocks that contain any valid (unpadded) key
    valid = ~key_padding_mask  # [B, S]
    kb_cap = 0
    for b in range(B):
        nz = np.nonzero(valid[b])[0]
        cap = (int(nz.max()) // 128 + 1) if nz.size else 1
        kb_cap = max(kb_cap, cap)

    scale = np.float32(1.0 / np.sqrt(DK))
    m01 = (np.arange(128)[None, :] >= np.arange(128)[:, None]).astype(np.float16)
    mask01 = np.ascontiguousarray(np.stack([m01, m01], axis=1))  # [128, 2, 128]

    # per-batch prepacked activations (shared by the batch's 4 cores)
    xq_b = [_prepack_x(query[:, b, :].T.astype(np.float16)) for b in range(B)]
    xk_b = [_prepack_x(key[:, b, :].T.astype(np.float16)) for b in range(B)]
    xv_b = [_prepack_x(value[:, b, :].T.astype(np.float16)) for b in range(B)]
    pad_b = [
        np.where(key_padding_mask[b], NEG, 0.0).astype(np.float32).reshape(KB, 128).T
        for b in range(B)
    ]

    in_maps = []
    for c in range(N_CORES):
        b = c // GROUPS
        g = c % GROUPS
        o0 = g * OC
        osl = slice(o0, o0 + OC)
        consts = np.zeros((128, 20), dtype=np.float32)
        consts[:, 0:2] = (bq[osl] * scale).reshape(OT, 128).T
        consts[:, 2:4] = bk[osl].reshape(OT, 128).T
        consts[:, 4:20] = pad_b[b]
        in_maps.append(
            {
                "xq": xq_b[b],
                "xk": xk_b[b],
                "xv": xv_b[b],
                "wq": np.ascontiguousarray(
                    (Wq[osl, :] * scale).T.astype(np.float16)
                    .reshape(IT, 128, OC).transpose(1, 0, 2)
                ),
                "wk": np.ascontiguousarray(
                    Wk[osl, :].T.astype(np.float16)
                    .reshape(IT, 128, OC).transpose(1, 0, 2)
                ),
                "wv": np.ascontiguousarray(
                    Wv[osl, :].T.astype(np.float16)
                    .reshape(IT, 128, OC).transpose(1, 0, 2)
                ),
                "wo": np.ascontiguousarray(
                    Wo[:, osl].T.astype(np.float16)
                    .reshape(OT, 128, D).transpose(1, 0, 2)
                ),
                "consts": consts,
                "mask01": mask01,
            }
        )

    res = bass_utils.run_bass_kernel_spmd(
        _get_nc(kb_cap), in_maps, core_ids=list(range(N_CORES))
    )
    _cache["last_res"] = res

    bo2 = bo + Wo @ bv  # softmax weights sum to 1: attn(v+bv) = attn(v)+bv
    out = np.zeros((S, B, D), dtype=np.float32)
    for b in range(B):
        acc = np.zeros((D, S), dtype=np.float32)
        for g in range(GROUPS):
            acc += res.results[b * GROUPS + g]["out_t"].astype(np.float32)
        out[:, b, :] = acc.T + bo2[None, :]
    return out
